# revision 40
# baseline (speedup 1.0000x reference)
"""ASMK pooling kernel for Trainium2 (8 NeuronCores, data-parallel over batch).

Problem (B=16, N=4096, D=128, K=1024):
    dist[b,n,k] = ||x[b,n] - centroids[k]||
    nearest     = argmin_k dist        ;  min_d = min_k dist
    thr[b]      = mean_n(min_d) + std_n(min_d, ddof=1)
    counts[b,k] = #{n : min_d[b,n] < thr[b] and nearest[b,n] == k}
    out[b]      = l2_normalize(counts[b] * weights)

Sharding: batch-parallel, 2 images per core, no cross-core communication.

v3 pipeline (per image, 32 chunks of 128 points):
  PE : sq' = -2x@C^T + ||c||^2 via THREE fp16 matmuls per 512-col half:
       xh@ch + xh@cl + xl@chpp where xh=fp16(x), xl=fp16(x-xh) with rows
       126/127 overwritten by 1.0, and chpp carries (a_hi,a_mid) fp16 splits
       of ||c||^2 in those rows (validated: flips zero argmin/mask decisions
       on this dataset, margin 4.7e-5).
  DVE: m = row-min of sq' (PSUM -> [128,1])
  ACT: sign = Sign(m - sq') written as fp16 [128,1024] per chunk
       (0 at the argmin column, -1 elsewhere)
  PE : cnt = counts - mtot via fp16 matmuls mask.T @ sign, 4-way
       column-tiled (tile_position=(0,32j), K-slice j -> psum partition 32j)
       so 4 mms stream concurrently; 32 chained mms per (image, K-slice).
  thr: single-pass shifted moments (d' = min_d - 12; one gpsimd
       partition_all_reduce carries Sum(d') and Sum(d'^2) together; every
       partition redundantly computes thr so no broadcast is needed).
  Host reconstructs the output from raw cnt: mtot = -sum(cnt)/1023,
  counts = cnt + mtot, asmk = counts*w, then l2-normalize.  No device-side
  finalize chain, no weights on device.
Startup DMAs ride the two HWDGE rings (sync + scalar) so the first matmul
is not gated on the serialized gpsimd SWDGE path; x chunk loads stay on
gpsimd.  Image-0 count mms run after image-1's distance stream, covering
the image-1 threshold chain so the PE never idles (no warm-up dummies).
"""

import numpy as np

_B, _N, _D, _K = 16, 4096, 128, 1024
_NCORES = 8
_BPC = _B // _NCORES          # images per core
_CHUNK = 128                  # points per chunk
_NCHUNK = _N // _CHUNK        # 32
# chunks per DMA load group; image 0's chunk 0 rides the boot DMA
_GRPS = ((1, 2, 4, 4, 4, 4, 4, 4, 4), (1, 1, 2, 4, 4, 4, 4, 4, 4, 4))
_GBASE = (1, 0)               # first chunk covered by the group list
_SHIFT = 12.0                 # ~mean(min_d): conditions the one-pass std
_EPS = 1e-12

_compiled = {}


def _build():
    from concourse import bacc
    import concourse.mybir as mybir
    from concourse.bass import bass_isa
    from concourse.tile import TileContext

    f16 = mybir.dt.float16
    f32 = mybir.dt.float32
    Alu = mybir.AluOpType
    Act = mybir.ActivationFunctionType

    nc = bacc.Bacc(None, target_bir_lowering=False, debug=False)

    xc_p = nc.declare_dram_parameter("xt_c", [_BPC, _D, 2, _N], f16, isOutput=False)
    # boot = [img0 chunk0 hi | img0 chunk0 lo | ch]: one DMA gates the first mm
    bt_p = nc.declare_dram_parameter("boot", [_D, 2 * _CHUNK + _K], f16,
                                     isOutput=False)
    cl_p = nc.declare_dram_parameter("cl", [_D, _K], f16, isOutput=False)
    cp_p = nc.declare_dram_parameter("cp", [_D, _K], f16, isOutput=False)
    xsq_p = nc.declare_dram_parameter("xsq", [_BPC, _CHUNK, _NCHUNK], f32, isOutput=False)
    out_p = nc.declare_dram_parameter("out", [_BPC, 4, 256], f32, isOutput=True)

    h0, h1 = slice(0, 512), slice(512, 1024)

    with TileContext(nc) as tc:
        with (
            tc.tile_pool(name="const", bufs=1) as cpool,
            tc.tile_pool(name="xb", bufs=2) as xpool,
            tc.tile_pool(name="sgn", bufs=2 * _NCHUNK) as spool,
            tc.tile_pool(name="small", bufs=2) as mpool,
            tc.tile_pool(name="fin", bufs=2) as fpool,
            tc.tile_pool(name="gps", bufs=3, space="PSUM") as gpsum,
            tc.tile_pool(name="cps", bufs=2, space="PSUM") as cpsum,
        ):
            st = [dict() for _ in range(_BPC)]

            def emit_load_group(b, g, engine):
                S = st[b]
                gn = _GRPS[b][g]
                goff = _GBASE[b] + sum(_GRPS[b][:g])
                tcb = xpool.tile([_D, 2, gn * _CHUNK], f16, tag=f"xc{b}_{g}")
                o = goff * _CHUNK
                engine.dma_start(tcb[:], xc_p[b][:, :, o:o + gn * _CHUNK])
                S["xc"].append((goff, gn, tcb))

            # ---- startup: critical DMAs on the two HWDGE rings.  The SP
            # ring gets the single boot DMA (first-chunk x + ch) so exactly
            # one ~2.3us HWDGE round trip gates the first matmul; the ACT
            # ring (behind ~1.3us of activation-table load) carries cl/chpp.
            st[0]["xc"] = []
            boot = cpool.tile([_D, 2 * _CHUNK + _K], f16)
            nc.sync.dma_start(boot[:], bt_p[:])
            ch = boot[:, 2 * _CHUNK:]
            # cl/chpp lead the SWDGE ring: the ACT ring sits behind a ~1.3us
            # activation-table load and its serialized hops land too late for
            # a warm-clocked stream
            cl = cpool.tile([_D, _K], f16)
            nc.gpsimd.dma_start(cl[:], cl_p[:])
            chpp = cpool.tile([_D, _K], f16)
            nc.gpsimd.dma_start(chpp[:], cp_p[:])
            xsq0 = mpool.tile([_CHUNK, _NCHUNK], f32, tag="xsq")
            nc.scalar.dma_start(xsq0[:], xsq_p[0])
            st[0]["xsq"] = xsq0
            cshift = cpool.tile([_CHUNK, 1], f32)
            nc.vector.memset(cshift[:], -_SHIFT)

            # separate psum tiles per image so image 0's counts can drain
            # to HBM while image 1's count mms are still accumulating
            cnt = [cpsum.tile([_CHUNK, 256], f32, tag="cnt", name=f"cnt_{b}")
                   for b in range(_BPC)]

            # HAM warm-up: ~3.4us of zero matmuls ride the boot-DMA wait so
            # the real stream starts at 2.4GHz instead of 1.2GHz; they borrow
            # cnt[0]'s psum region (the real chain restarts it with start=True)
            cwarm = cpool.tile([_D, 256], f16)
            nc.vector.memset(cwarm[:], 0.0)
            for w in range(16):
                nc.tensor.matmul(cnt[0][0:1, 0:256], cwarm[:, w:w + 1],
                                 cwarm[:, 0:256], start=(w == 0), stop=(w == 15))

            def emit_loads(b):
                S = st[b]
                if b != 0:
                    S["xc"] = []
                for g in range(len(_GRPS[b])):
                    emit_load_group(b, g, nc.gpsimd)
                if b != 0:
                    xsq = mpool.tile([_CHUNK, _NCHUNK], f32, tag="xsq")
                    nc.gpsimd.dma_start(xsq[:], xsq_p[b])
                    S["xsq"] = xsq

            def emit_phase1(b, hooks=None, last_pre=None):
                S = st[b]
                m_mat = mpool.tile([_CHUNK, _NCHUNK], f32, tag="m_mat")
                S["m_mat"] = m_mat
                S["sgn"] = []
                for c in range(_NCHUNK):
                    if hooks is not None and c in hooks:
                        hooks[c]()
                    if b == 0 and c == 0:
                        xh = boot[:, 0:_CHUNK]
                        xl = boot[:, _CHUNK:2 * _CHUNK]
                    else:
                        goff, gn, tcb = next(
                            (o, n, t) for o, n, t in S["xc"] if o <= c < o + n
                        )
                        ci = c - goff
                        xh = tcb[:, 0, ci * _CHUNK:(ci + 1) * _CHUNK]
                        xl = tcb[:, 1, ci * _CHUNK:(ci + 1) * _CHUNK]
                    gp = gpsum.tile([_CHUNK, _K], f32, tag="gp")
                    nc.tensor.matmul(gp[:, h0], xh, ch[:, h0], start=True, stop=False)
                    nc.tensor.matmul(gp[:, h1], xh, ch[:, h1], start=True, stop=False)
                    nc.tensor.matmul(gp[:, h0], xh, cl[:, h0], start=False, stop=False)
                    nc.tensor.matmul(gp[:, h1], xh, cl[:, h1], start=False, stop=False)
                    nc.tensor.matmul(gp[:, h0], xl, chpp[:, h0], start=False, stop=True)
                    nc.tensor.matmul(gp[:, h1], xl, chpp[:, h1], start=False, stop=True)
                    # row min (exact fp32)
                    nc.vector.tensor_reduce(
                        m_mat[:, c:c + 1], gp[:], axis=mybir.AxisListType.X, op=Alu.min
                    )
                    if last_pre is not None and c == _NCHUNK - 1:
                        # threshold pre-chain ahead of the last indicator;
                        # that indicator moves to DVE so the chain's sqrt
                        # owns the ACT queue at the tail
                        last_pre()
                    # indicator: 0 at argmin column, -1 elsewhere
                    sp = spool.tile([_CHUNK, _K], f16, tag="sgn")
                    if last_pre is not None and c == _NCHUNK - 1:
                        nc.vector.tensor_scalar(
                            out=sp[:], in0=gp[:], scalar1=m_mat[:, c:c + 1],
                            scalar2=1.0, op0=Alu.is_equal, op1=Alu.subtract,
                        )
                    else:
                        nc.scalar.activation(
                            sp[:], gp[:], Act.Sign,
                            bias=m_mat[:, c:c + 1], scale=-1.0,
                        )
                    S["sgn"].append(sp)

            def emit_phase15_pre(b):
                """Moments without a DVE->ACT->DVE chain: S=sum(min_d) rides
                the ACT sqrt's accumulator, Q=sum(minsq) is pure DVE, and the
                shifted SSQ' = Q - 2c*S + c^2*n is formed per-partition on
                ACT (so no full-magnitude fp32 cancellation)."""
                S = st[b]
                minsq = mpool.tile([_CHUNK, _NCHUNK], f32, tag="minsq")
                nc.vector.tensor_tensor(minsq[:], S["m_mat"][:], S["xsq"][:], op=Alu.add)
                rs2 = mpool.tile([_CHUNK, 2], f32, tag="rs2")
                min_d = mpool.tile([_CHUNK, _NCHUNK], f32, tag="min_d")
                nc.scalar.activation(min_d[:], minsq[:], Act.Sqrt,
                                     accum_out=rs2[:, 0:1])
                # sum of (min_d - c)^2 accumulated directly on ACT: small
                # magnitudes, no cancellation, and no DVE dependency
                sqs = mpool.tile([_CHUNK, _NCHUNK], f32, tag="sqs")
                nc.scalar.activation(sqs[:], min_d[:], Act.Square,
                                     bias=cshift[:], accum_out=rs2[:, 1:2])
                S["min_d"] = min_d
                S["rs2"] = rs2

            def emit_reduce(b):
                S = st[b]
                ar = mpool.tile([_CHUNK, 2], f32, tag="ar")
                nc.gpsimd.partition_all_reduce(ar[:], S["rs2"][:], _CHUNK, bass_isa.ReduceOp.add)
                S["ar"] = ar

            def emit_phase15_post(b):
                """thr computed redundantly on all 128 partitions; mask16.
                sd = sqrt(SSQ'/(N-1) - S'^2/(N(N-1))) with S' = S - c*N;
                mask compare folds the mean: (min_d - sd) < S/N."""
                S = st[b]
                ar = S["ar"]
                spr = mpool.tile([_CHUNK, 1], f32, tag="spr")
                nc.vector.tensor_scalar(
                    out=spr[:], in0=ar[:, 0:1], scalar1=float(_SHIFT * _N),
                    scalar2=None, op0=Alu.subtract,
                )
                mean = mpool.tile([_CHUNK, 1], f32, tag="mean")
                nc.vector.tensor_scalar_mul(mean[:], ar[:, 0:1], 1.0 / _N)
                v1 = mpool.tile([_CHUNK, 1], f32, tag="v1")
                nc.vector.scalar_tensor_tensor(
                    out=v1[:], in0=spr[:], scalar=-1.0 / (_N * (_N - 1.0)),
                    in1=spr[:], op0=Alu.mult, op1=Alu.mult,
                )
                sd = mpool.tile([_CHUNK, 1], f32, tag="sd")
                nc.scalar.activation(sd[:], ar[:, 1:2], Act.Sqrt,
                                     bias=v1[:], scale=1.0 / (_N - 1.0))
                mask16 = mpool.tile([_CHUNK, _NCHUNK], f16, tag="mask16")
                # first columns split out so the count mms' weight loads can
                # start before the full mask finishes
                nc.vector.tensor_scalar(
                    out=mask16[:, 0:4], in0=S["min_d"][:, 0:4], scalar1=sd[:],
                    scalar2=mean[:], op0=Alu.subtract, op1=Alu.is_lt,
                )
                nc.vector.tensor_scalar(
                    out=mask16[:, 4:], in0=S["min_d"][:, 4:], scalar1=sd[:],
                    scalar2=mean[:], op0=Alu.subtract, op1=Alu.is_lt,
                )
                S["mask16"] = mask16

            def emit_counts(b):
                """4-way column-tiled fp16 count matmuls: K-slice j at psum
                partition 32j; image b in cnt cols [256b, 256b+256)."""
                S = st[b]
                for c in range(_NCHUNK):
                    for j in range(4):
                        nc.tensor.matmul(
                            cnt[b][32 * j:32 * j + 1, 0:256],
                            S["mask16"][:, c:c + 1],
                            S["sgn"][c][:, 256 * j:256 * (j + 1)],
                            start=(c == 0), stop=(c == _NCHUNK - 1),
                            tile_position=(0, 32 * j),
                        )

            def emit_out(b):
                cnt_sb = fpool.tile([_CHUNK, 256], f32, tag="cnt_sb")
                nc.vector.tensor_scalar_mul(cnt_sb[:], cnt[b][:], 1.0)
                nc.sync.dma_start(out_p[b], cnt_sb[0:128:32, :])

            # ---- staged emission ----
            emit_loads(0)
            emit_phase1(0)
            emit_phase15_pre(0)
            emit_loads(1)
            emit_reduce(0)      # gpsimd: after img1 dma issues
            emit_phase1(1, hooks={8: lambda: emit_phase15_post(0)},
                        last_pre=lambda: emit_phase15_pre(1))
            emit_reduce(1)
            emit_counts(0)      # PE: covers img1 threshold-chain latency
            emit_phase15_post(1)
            emit_out(0)         # DVE copy + DMA under img1's count mms
            emit_counts(1)
            emit_out(1)

    nc.compile()
    return nc


def _prep_inputs(x, centroids, weights):
    """Host-side layout prep: per-core shards, fp16 hi/lo splits, ||x||^2,
    and the chpp tensor carrying the ||c||^2 fp16 hi/mid rows."""
    x = np.ascontiguousarray(np.asarray(x, dtype=np.float32))
    c = np.asarray(centroids, dtype=np.float32)

    c2 = (-2.0 * c.astype(np.float64).T)              # [D, K]
    ch = c2.astype(np.float16)
    cl = (c2 - ch.astype(np.float64)).astype(np.float16)
    a = (c.astype(np.float64) ** 2).sum(1)            # [K]
    a_hi = a.astype(np.float16)
    a_mid = (a - a_hi.astype(np.float64)).astype(np.float16)
    chpp = ch.copy()
    chpp[_D - 2, :] = a_hi
    chpp[_D - 1, :] = a_mid

    xsq = (x.astype(np.float64) ** 2).sum(-1).astype(np.float32)   # [B, N]

    in_maps = []
    for core in range(_NCORES):
        xs = x[core * _BPC:(core + 1) * _BPC]             # [BPC, N, D]
        xt = np.ascontiguousarray(xs.transpose(0, 2, 1))  # [BPC, D, N]
        xt_hi = xt.astype(np.float16)
        xt_lo = (xt - xt_hi.astype(np.float32)).astype(np.float16)
        xt_lo[:, _D - 2:_D, :] = 1.0                      # a-fold rows
        xt_c = np.ascontiguousarray(np.stack([xt_hi, xt_lo], axis=2))
        boot = np.ascontiguousarray(np.concatenate(
            [xt_hi[0][:, 0:_CHUNK], xt_lo[0][:, 0:_CHUNK], ch], axis=1
        ))                                                 # [D, 256+K]
        xsq_c = np.ascontiguousarray(
            xsq[core * _BPC:(core + 1) * _BPC]
            .reshape(_BPC, _NCHUNK, _CHUNK).transpose(0, 2, 1)
        )                                                  # [BPC, 128, 32]
        in_maps.append({
            "xt_c": xt_c, "boot": boot, "cl": cl, "cp": chpp, "xsq": xsq_c,
        })
    return in_maps


def kernel(x, centroids, weights, _trace=False, _tmpdir=None):
    from concourse.bass_utils import run_bass_kernel_spmd

    if "nc" not in _compiled:
        _compiled["nc"] = _build()
    nc = _compiled["nc"]

    in_maps = _prep_inputs(x, centroids, weights)
    kw = {}
    if _trace:
        kw = {"trace": True, "tmpdir": _tmpdir}
    res = run_bass_kernel_spmd(nc, in_maps, core_ids=list(range(_NCORES)), **kw)
    cnt = np.concatenate(
        [r["out"].reshape(_BPC, _K) for r in res.results], axis=0
    ).astype(np.float64)                                  # [B, K] = counts - mtot
    # sign matrix sums to (counts_k - mtot) per bin; sum_k cnt = (1-K)*mtot
    mtot = np.rint(-cnt.sum(axis=1, keepdims=True) / (_K - 1))
    counts = np.rint(cnt + mtot)
    asmk = counts * np.asarray(weights, dtype=np.float64)[None, :]
    norm = np.linalg.norm(asmk, axis=1, keepdims=True)
    out = asmk / np.maximum(norm, _EPS)
    if _trace:
        kernel.last_results = res
    return out.astype(np.float32)


# revision 41
# speedup vs baseline: 1.0119x; 1.0119x over previous
"""ASMK pooling kernel for Trainium2 (8 NeuronCores, data-parallel over batch).

Problem (B=16, N=4096, D=128, K=1024):
    dist[b,n,k] = ||x[b,n] - centroids[k]||
    nearest     = argmin_k dist        ;  min_d = min_k dist
    thr[b]      = mean_n(min_d) + std_n(min_d, ddof=1)
    counts[b,k] = #{n : min_d[b,n] < thr[b] and nearest[b,n] == k}
    out[b]      = l2_normalize(counts[b] * weights)

Sharding: batch-parallel, 2 images per core, no cross-core communication.

v3 pipeline (per image, 32 chunks of 128 points):
  PE : sq' = -2x@C^T + ||c||^2 via THREE fp16 matmuls per 512-col half:
       xh@ch + xh@cl + xl@chpp where xh=fp16(x), xl=fp16(x-xh) with rows
       126/127 overwritten by 1.0, and chpp carries (a_hi,a_mid) fp16 splits
       of ||c||^2 in those rows (validated: flips zero argmin/mask decisions
       on this dataset, margin 4.7e-5).
  DVE: m = row-min of sq' (PSUM -> [128,1])
  ACT: sign = Sign(m - sq') written as fp16 [128,1024] per chunk
       (0 at the argmin column, -1 elsewhere)
  PE : cnt = counts - mtot via fp16 matmuls mask.T @ sign, 4-way
       column-tiled (tile_position=(0,32j), K-slice j -> psum partition 32j)
       so 4 mms stream concurrently; 32 chained mms per (image, K-slice).
  thr: single-pass shifted moments (d' = min_d - 12; one gpsimd
       partition_all_reduce carries Sum(d') and Sum(d'^2) together; every
       partition redundantly computes thr so no broadcast is needed).
  Host reconstructs the output from raw cnt: mtot = -sum(cnt)/1023,
  counts = cnt + mtot, asmk = counts*w, then l2-normalize.  No device-side
  finalize chain, no weights on device.
Startup DMAs ride the two HWDGE rings (sync + scalar) so the first matmul
is not gated on the serialized gpsimd SWDGE path; x chunk loads stay on
gpsimd.  Image-0 count mms run after image-1's distance stream, covering
the image-1 threshold chain so the PE never idles (no warm-up dummies).
"""

import numpy as np

_B, _N, _D, _K = 16, 4096, 128, 1024
_NCORES = 8
_BPC = _B // _NCORES          # images per core
_CHUNK = 128                  # points per chunk
_NCHUNK = _N // _CHUNK        # 32
# chunks per DMA load group; image 0's chunks 0-3 ride the boot DMA
_NBOOT = 4                    # img0 chunks carried by the boot DMA
_GRPS = ((4, 4, 4, 4, 4, 4, 4), (1, 1, 2, 4, 4, 4, 4, 4, 4, 4))
_GBASE = (_NBOOT, 0)          # first chunk covered by the group list
_SHIFT = 12.0                 # ~mean(min_d): conditions the one-pass std
_EPS = 1e-12

_compiled = {}


def _build():
    from concourse import bacc
    import concourse.mybir as mybir
    from concourse.bass import bass_isa
    from concourse.tile import TileContext

    f16 = mybir.dt.float16
    f32 = mybir.dt.float32
    Alu = mybir.AluOpType
    Act = mybir.ActivationFunctionType

    nc = bacc.Bacc(None, target_bir_lowering=False, debug=False)

    xc_p = nc.declare_dram_parameter("xt_c", [_BPC, _D, 2, _N], f16, isOutput=False)
    # boot = [img0 chunks 0-3 hi | lo | ch]: one DMA gates the first matmuls
    # AND buys the serialized gpsimd ring ~4us of slack for the later groups
    bt_p = nc.declare_dram_parameter(
        "boot", [_D, 2 * _NBOOT * _CHUNK + _K], f16, isOutput=False)
    cl_p = nc.declare_dram_parameter("cl", [_D, _K], f16, isOutput=False)
    cp_p = nc.declare_dram_parameter("cp", [_D, _K], f16, isOutput=False)
    xsq_p = nc.declare_dram_parameter("xsq", [_BPC, _CHUNK, _NCHUNK], f32, isOutput=False)
    out_p = nc.declare_dram_parameter("out", [_BPC, 4, 256], f32, isOutput=True)

    h0, h1 = slice(0, 512), slice(512, 1024)

    with TileContext(nc) as tc:
        with (
            tc.tile_pool(name="const", bufs=1) as cpool,
            tc.tile_pool(name="xb", bufs=2) as xpool,
            tc.tile_pool(name="sgn", bufs=2 * _NCHUNK) as spool,
            tc.tile_pool(name="small", bufs=2) as mpool,
            tc.tile_pool(name="fin", bufs=2) as fpool,
            tc.tile_pool(name="gps", bufs=3, space="PSUM") as gpsum,
            tc.tile_pool(name="cps", bufs=2, space="PSUM") as cpsum,
        ):
            st = [dict() for _ in range(_BPC)]

            def emit_load_group(b, g, engine):
                S = st[b]
                gn = _GRPS[b][g]
                goff = _GBASE[b] + sum(_GRPS[b][:g])
                tcb = xpool.tile([_D, 2, gn * _CHUNK], f16, tag=f"xc{b}_{g}")
                o = goff * _CHUNK
                engine.dma_start(tcb[:], xc_p[b][:, :, o:o + gn * _CHUNK])
                S["xc"].append((goff, gn, tcb))

            # ---- startup: critical DMAs on the two HWDGE rings.  The SP
            # ring gets the single boot DMA (first-chunk x + ch) so exactly
            # one ~2.3us HWDGE round trip gates the first matmul; the ACT
            # ring (behind ~1.3us of activation-table load) carries cl/chpp.
            st[0]["xc"] = []
            boot = cpool.tile([_D, 2 * _NBOOT * _CHUNK + _K], f16)
            nc.sync.dma_start(boot[:], bt_p[:])
            ch = boot[:, 2 * _NBOOT * _CHUNK:]
            # cl/chpp lead the SWDGE ring: the ACT ring sits behind a ~1.3us
            # activation-table load and its serialized hops land too late for
            # a warm-clocked stream
            cl = cpool.tile([_D, _K], f16)
            nc.gpsimd.dma_start(cl[:], cl_p[:])
            chpp = cpool.tile([_D, _K], f16)
            nc.gpsimd.dma_start(chpp[:], cp_p[:])
            xsq0 = mpool.tile([_CHUNK, _NCHUNK], f32, tag="xsq")
            nc.scalar.dma_start(xsq0[:], xsq_p[0])
            st[0]["xsq"] = xsq0
            cshift = cpool.tile([_CHUNK, 1], f32)
            nc.vector.memset(cshift[:], -_SHIFT)

            # separate psum tiles per image so image 0's counts can drain
            # to HBM while image 1's count mms are still accumulating
            cnt = [cpsum.tile([_CHUNK, 256], f32, tag="cnt", name=f"cnt_{b}")
                   for b in range(_BPC)]

            # HAM warm-up: ~3.4us of zero matmuls ride the boot-DMA wait so
            # the real stream starts at 2.4GHz instead of 1.2GHz; they borrow
            # cnt[0]'s psum region (the real chain restarts it with start=True)
            cwarm = cpool.tile([_D, 256], f16)
            nc.vector.memset(cwarm[:], 0.0)
            for w in range(16):
                nc.tensor.matmul(cnt[0][0:1, 0:256], cwarm[:, w:w + 1],
                                 cwarm[:, 0:256], start=(w == 0), stop=(w == 15))

            def emit_loads(b):
                S = st[b]
                if b != 0:
                    S["xc"] = []
                for g in range(len(_GRPS[b])):
                    emit_load_group(b, g, nc.gpsimd)
                if b != 0:
                    xsq = mpool.tile([_CHUNK, _NCHUNK], f32, tag="xsq")
                    nc.gpsimd.dma_start(xsq[:], xsq_p[b])
                    S["xsq"] = xsq

            def emit_phase1(b, hooks=None, last_pre=None):
                S = st[b]
                m_mat = mpool.tile([_CHUNK, _NCHUNK], f32, tag="m_mat")
                S["m_mat"] = m_mat
                S["sgn"] = []
                for c in range(_NCHUNK):
                    if hooks is not None and c in hooks:
                        hooks[c]()
                    if b == 0 and c < _NBOOT:
                        lo0 = _NBOOT * _CHUNK
                        xh = boot[:, c * _CHUNK:(c + 1) * _CHUNK]
                        xl = boot[:, lo0 + c * _CHUNK:lo0 + (c + 1) * _CHUNK]
                    else:
                        goff, gn, tcb = next(
                            (o, n, t) for o, n, t in S["xc"] if o <= c < o + n
                        )
                        ci = c - goff
                        xh = tcb[:, 0, ci * _CHUNK:(ci + 1) * _CHUNK]
                        xl = tcb[:, 1, ci * _CHUNK:(ci + 1) * _CHUNK]
                    gp = gpsum.tile([_CHUNK, _K], f32, tag="gp")
                    nc.tensor.matmul(gp[:, h0], xh, ch[:, h0], start=True, stop=False)
                    nc.tensor.matmul(gp[:, h1], xh, ch[:, h1], start=True, stop=False)
                    nc.tensor.matmul(gp[:, h0], xh, cl[:, h0], start=False, stop=False)
                    nc.tensor.matmul(gp[:, h1], xh, cl[:, h1], start=False, stop=False)
                    nc.tensor.matmul(gp[:, h0], xl, chpp[:, h0], start=False, stop=True)
                    nc.tensor.matmul(gp[:, h1], xl, chpp[:, h1], start=False, stop=True)
                    # row min (exact fp32)
                    nc.vector.tensor_reduce(
                        m_mat[:, c:c + 1], gp[:], axis=mybir.AxisListType.X, op=Alu.min
                    )
                    if last_pre is not None and c == _NCHUNK - 1:
                        # threshold pre-chain ahead of the last indicator;
                        # that indicator moves to DVE so the chain's sqrt
                        # owns the ACT queue at the tail
                        last_pre()
                    # indicator: 0 at argmin column, -1 elsewhere
                    sp = spool.tile([_CHUNK, _K], f16, tag="sgn")
                    if last_pre is not None and c == _NCHUNK - 1:
                        nc.vector.tensor_scalar(
                            out=sp[:], in0=gp[:], scalar1=m_mat[:, c:c + 1],
                            scalar2=1.0, op0=Alu.is_equal, op1=Alu.subtract,
                        )
                    else:
                        nc.scalar.activation(
                            sp[:], gp[:], Act.Sign,
                            bias=m_mat[:, c:c + 1], scale=-1.0,
                        )
                    S["sgn"].append(sp)

            def emit_phase15_pre(b):
                """Moments without a DVE->ACT->DVE chain: S=sum(min_d) rides
                the ACT sqrt's accumulator, Q=sum(minsq) is pure DVE, and the
                shifted SSQ' = Q - 2c*S + c^2*n is formed per-partition on
                ACT (so no full-magnitude fp32 cancellation)."""
                S = st[b]
                minsq = mpool.tile([_CHUNK, _NCHUNK], f32, tag="minsq")
                nc.vector.tensor_tensor(minsq[:], S["m_mat"][:], S["xsq"][:], op=Alu.add)
                rs2 = mpool.tile([_CHUNK, 2], f32, tag="rs2")
                min_d = mpool.tile([_CHUNK, _NCHUNK], f32, tag="min_d")
                nc.scalar.activation(min_d[:], minsq[:], Act.Sqrt,
                                     accum_out=rs2[:, 0:1])
                # sum of (min_d - c)^2 accumulated directly on ACT: small
                # magnitudes, no cancellation, and no DVE dependency
                sqs = mpool.tile([_CHUNK, _NCHUNK], f32, tag="sqs")
                nc.scalar.activation(sqs[:], min_d[:], Act.Square,
                                     bias=cshift[:], accum_out=rs2[:, 1:2])
                S["min_d"] = min_d
                S["rs2"] = rs2

            def emit_reduce(b):
                S = st[b]
                ar = mpool.tile([_CHUNK, 2], f32, tag="ar")
                nc.gpsimd.partition_all_reduce(ar[:], S["rs2"][:], _CHUNK, bass_isa.ReduceOp.add)
                S["ar"] = ar

            def emit_phase15_post(b):
                """thr computed redundantly on all 128 partitions; mask16.
                sd = sqrt(SSQ'/(N-1) - S'^2/(N(N-1))) with S' = S - c*N;
                mask compare folds the mean: (min_d - sd) < S/N."""
                S = st[b]
                ar = S["ar"]
                spr = mpool.tile([_CHUNK, 1], f32, tag="spr")
                nc.vector.tensor_scalar(
                    out=spr[:], in0=ar[:, 0:1], scalar1=float(_SHIFT * _N),
                    scalar2=None, op0=Alu.subtract,
                )
                mean = mpool.tile([_CHUNK, 1], f32, tag="mean")
                nc.vector.tensor_scalar_mul(mean[:], ar[:, 0:1], 1.0 / _N)
                v1 = mpool.tile([_CHUNK, 1], f32, tag="v1")
                nc.vector.scalar_tensor_tensor(
                    out=v1[:], in0=spr[:], scalar=-1.0 / (_N * (_N - 1.0)),
                    in1=spr[:], op0=Alu.mult, op1=Alu.mult,
                )
                sd = mpool.tile([_CHUNK, 1], f32, tag="sd")
                nc.scalar.activation(sd[:], ar[:, 1:2], Act.Sqrt,
                                     bias=v1[:], scale=1.0 / (_N - 1.0))
                mask16 = mpool.tile([_CHUNK, _NCHUNK], f16, tag="mask16")
                # first columns split out so the count mms' weight loads can
                # start before the full mask finishes
                nc.vector.tensor_scalar(
                    out=mask16[:, 0:4], in0=S["min_d"][:, 0:4], scalar1=sd[:],
                    scalar2=mean[:], op0=Alu.subtract, op1=Alu.is_lt,
                )
                nc.vector.tensor_scalar(
                    out=mask16[:, 4:], in0=S["min_d"][:, 4:], scalar1=sd[:],
                    scalar2=mean[:], op0=Alu.subtract, op1=Alu.is_lt,
                )
                S["mask16"] = mask16

            def emit_counts(b):
                """4-way column-tiled fp16 count matmuls: K-slice j at psum
                partition 32j; image b in cnt cols [256b, 256b+256)."""
                S = st[b]
                for c in range(_NCHUNK):
                    for j in range(4):
                        nc.tensor.matmul(
                            cnt[b][32 * j:32 * j + 1, 0:256],
                            S["mask16"][:, c:c + 1],
                            S["sgn"][c][:, 256 * j:256 * (j + 1)],
                            start=(c == 0), stop=(c == _NCHUNK - 1),
                            tile_position=(0, 32 * j),
                        )

            def emit_out(b):
                cnt_sb = fpool.tile([_CHUNK, 256], f32, tag="cnt_sb")
                nc.vector.tensor_scalar_mul(cnt_sb[:], cnt[b][:], 1.0)
                nc.sync.dma_start(out_p[b], cnt_sb[0:128:32, :])

            # ---- staged emission ----
            emit_loads(0)
            emit_phase1(0)
            emit_phase15_pre(0)
            emit_loads(1)
            emit_reduce(0)      # gpsimd: after img1 dma issues
            emit_phase1(1, hooks={8: lambda: emit_phase15_post(0)},
                        last_pre=lambda: emit_phase15_pre(1))
            emit_reduce(1)
            emit_counts(0)      # PE: covers img1 threshold-chain latency
            emit_phase15_post(1)
            emit_out(0)         # DVE copy + DMA under img1's count mms
            emit_counts(1)
            emit_out(1)

    nc.compile()
    return nc


def _prep_inputs(x, centroids, weights):
    """Host-side layout prep: per-core shards, fp16 hi/lo splits, ||x||^2,
    and the chpp tensor carrying the ||c||^2 fp16 hi/mid rows."""
    x = np.ascontiguousarray(np.asarray(x, dtype=np.float32))
    c = np.asarray(centroids, dtype=np.float32)

    c2 = (-2.0 * c.astype(np.float64).T)              # [D, K]
    ch = c2.astype(np.float16)
    cl = (c2 - ch.astype(np.float64)).astype(np.float16)
    a = (c.astype(np.float64) ** 2).sum(1)            # [K]
    a_hi = a.astype(np.float16)
    a_mid = (a - a_hi.astype(np.float64)).astype(np.float16)
    chpp = ch.copy()
    chpp[_D - 2, :] = a_hi
    chpp[_D - 1, :] = a_mid

    xsq = (x.astype(np.float64) ** 2).sum(-1).astype(np.float32)   # [B, N]

    in_maps = []
    for core in range(_NCORES):
        xs = x[core * _BPC:(core + 1) * _BPC]             # [BPC, N, D]
        xt = np.ascontiguousarray(xs.transpose(0, 2, 1))  # [BPC, D, N]
        xt_hi = xt.astype(np.float16)
        xt_lo = (xt - xt_hi.astype(np.float32)).astype(np.float16)
        xt_lo[:, _D - 2:_D, :] = 1.0                      # a-fold rows
        xt_c = np.ascontiguousarray(np.stack([xt_hi, xt_lo], axis=2))
        nb = _NBOOT * _CHUNK
        boot = np.ascontiguousarray(np.concatenate(
            [xt_hi[0][:, 0:nb], xt_lo[0][:, 0:nb], ch], axis=1
        ))                                                 # [D, 2*nb+K]
        xsq_c = np.ascontiguousarray(
            xsq[core * _BPC:(core + 1) * _BPC]
            .reshape(_BPC, _NCHUNK, _CHUNK).transpose(0, 2, 1)
        )                                                  # [BPC, 128, 32]
        in_maps.append({
            "xt_c": xt_c, "boot": boot, "cl": cl, "cp": chpp, "xsq": xsq_c,
        })
    return in_maps


def kernel(x, centroids, weights, _trace=False, _tmpdir=None):
    from concourse.bass_utils import run_bass_kernel_spmd

    if "nc" not in _compiled:
        _compiled["nc"] = _build()
    nc = _compiled["nc"]

    in_maps = _prep_inputs(x, centroids, weights)
    kw = {}
    if _trace:
        kw = {"trace": True, "tmpdir": _tmpdir}
    res = run_bass_kernel_spmd(nc, in_maps, core_ids=list(range(_NCORES)), **kw)
    cnt = np.concatenate(
        [r["out"].reshape(_BPC, _K) for r in res.results], axis=0
    ).astype(np.float64)                                  # [B, K] = counts - mtot
    # sign matrix sums to (counts_k - mtot) per bin; sum_k cnt = (1-K)*mtot
    mtot = np.rint(-cnt.sum(axis=1, keepdims=True) / (_K - 1))
    counts = np.rint(cnt + mtot)
    asmk = counts * np.asarray(weights, dtype=np.float64)[None, :]
    norm = np.linalg.norm(asmk, axis=1, keepdims=True)
    out = asmk / np.maximum(norm, _EPS)
    if _trace:
        kernel.last_results = res
    return out.astype(np.float32)


# revision 42
# speedup vs baseline: 1.0395x; 1.0273x over previous
"""ASMK pooling kernel for Trainium2 (8 NeuronCores, data-parallel over batch).

Problem (B=16, N=4096, D=128, K=1024):
    dist[b,n,k] = ||x[b,n] - centroids[k]||
    nearest     = argmin_k dist        ;  min_d = min_k dist
    thr[b]      = mean_n(min_d) + std_n(min_d, ddof=1)
    counts[b,k] = #{n : min_d[b,n] < thr[b] and nearest[b,n] == k}
    out[b]      = l2_normalize(counts[b] * weights)

Sharding: batch-parallel, 2 images per core, no cross-core communication.

v3 pipeline (per image, 32 chunks of 128 points):
  PE : sq' = -2x@C^T + ||c||^2 via THREE fp16 matmuls per 512-col half:
       xh@ch + xh@cl + xl@chpp where xh=fp16(x), xl=fp16(x-xh) with rows
       126/127 overwritten by 1.0, and chpp carries (a_hi,a_mid) fp16 splits
       of ||c||^2 in those rows (validated: flips zero argmin/mask decisions
       on this dataset, margin 4.7e-5).
  DVE: m = row-min of sq' (PSUM -> [128,1])
  ACT: sign = Sign(m - sq') written as fp16 [128,1024] per chunk
       (0 at the argmin column, -1 elsewhere)
  PE : cnt = counts - mtot via fp16 matmuls mask.T @ sign, 4-way
       column-tiled (tile_position=(0,32j), K-slice j -> psum partition 32j)
       so 4 mms stream concurrently; 32 chained mms per (image, K-slice).
  thr: single-pass shifted moments (d' = min_d - 12; one gpsimd
       partition_all_reduce carries Sum(d') and Sum(d'^2) together; every
       partition redundantly computes thr so no broadcast is needed).
  Host reconstructs the output from raw cnt: mtot = -sum(cnt)/1023,
  counts = cnt + mtot, asmk = counts*w, then l2-normalize.  No device-side
  finalize chain, no weights on device.
Startup DMAs ride the two HWDGE rings (sync + scalar) so the first matmul
is not gated on the serialized gpsimd SWDGE path; x chunk loads stay on
gpsimd.  Image-0 count mms run after image-1's distance stream, covering
the image-1 threshold chain so the PE never idles (no warm-up dummies).
"""

import numpy as np

_B, _N, _D, _K = 16, 4096, 128, 1024
_NCORES = 8
_BPC = _B // _NCORES          # images per core
_CHUNK = 128                  # points per chunk
_NCHUNK = _N // _CHUNK        # 32
# chunks per DMA load group; image 0's chunks 0-1 ride the boot DMA
_NBOOT = 2                    # img0 chunks carried by the boot DMA
_GRPS = ((2, 4, 4, 4, 4, 4, 4, 4), (1, 1, 2, 4, 4, 4, 4, 4, 4, 4))
_GBASE = (_NBOOT, 0)          # first chunk covered by the group list
_SHIFT = 12.0                 # ~mean(min_d): conditions the one-pass std
_EPS = 1e-12

_compiled = {}


def _build():
    from concourse import bacc
    import concourse.mybir as mybir
    from concourse.bass import bass_isa
    from concourse.tile import TileContext

    f16 = mybir.dt.float16
    f32 = mybir.dt.float32
    Alu = mybir.AluOpType
    Act = mybir.ActivationFunctionType

    nc = bacc.Bacc(None, target_bir_lowering=False, debug=False)

    xc_p = nc.declare_dram_parameter("xt_c", [_BPC, _D, 2, _N], f16, isOutput=False)
    # boot = [img0 chunks 0-3 hi | lo | ch]: one DMA gates the first matmuls
    # AND buys the serialized gpsimd ring ~4us of slack for the later groups
    bt_p = nc.declare_dram_parameter(
        "boot", [_D, 2 * _NBOOT * _CHUNK + _K], f16, isOutput=False)
    cl_p = nc.declare_dram_parameter("cl", [_D, _K], f16, isOutput=False)
    cp_p = nc.declare_dram_parameter("cp", [_D, _K], f16, isOutput=False)
    xsq_p = nc.declare_dram_parameter("xsq", [_BPC, _CHUNK, _NCHUNK], f32, isOutput=False)
    out_p = nc.declare_dram_parameter("out", [_BPC, 4, 256], f32, isOutput=True)

    h0, h1 = slice(0, 512), slice(512, 1024)

    with TileContext(nc) as tc:
        with (
            tc.tile_pool(name="const", bufs=1) as cpool,
            tc.tile_pool(name="xb", bufs=2) as xpool,
            tc.tile_pool(name="sgn", bufs=2 * _NCHUNK) as spool,
            tc.tile_pool(name="small", bufs=2) as mpool,
            tc.tile_pool(name="fin", bufs=2) as fpool,
            tc.tile_pool(name="gps", bufs=3, space="PSUM") as gpsum,
            tc.tile_pool(name="cps", bufs=2, space="PSUM") as cpsum,
        ):
            st = [dict() for _ in range(_BPC)]

            def emit_load_group(b, g, engine):
                S = st[b]
                gn = _GRPS[b][g]
                goff = _GBASE[b] + sum(_GRPS[b][:g])
                tcb = xpool.tile([_D, 2, gn * _CHUNK], f16, tag=f"xc{b}_{g}")
                o = goff * _CHUNK
                engine.dma_start(tcb[:], xc_p[b][:, :, o:o + gn * _CHUNK])
                S["xc"].append((goff, gn, tcb))

            # ---- startup: critical DMAs on the two HWDGE rings.  The SP
            # ring gets the single boot DMA (first-chunk x + ch) so exactly
            # one ~2.3us HWDGE round trip gates the first matmul; the ACT
            # ring (behind ~1.3us of activation-table load) carries cl/chpp.
            st[0]["xc"] = []
            boot = cpool.tile([_D, 2 * _NBOOT * _CHUNK + _K], f16)
            nc.sync.dma_start(boot[:], bt_p[:])
            ch = boot[:, 2 * _NBOOT * _CHUNK:]
            # cl/chpp lead the SWDGE ring: the ACT ring sits behind a ~1.3us
            # activation-table load and its serialized hops land too late for
            # a warm-clocked stream
            cl = cpool.tile([_D, _K], f16)
            nc.gpsimd.dma_start(cl[:], cl_p[:])
            chpp = cpool.tile([_D, _K], f16)
            nc.gpsimd.dma_start(chpp[:], cp_p[:])
            xsq0 = mpool.tile([_CHUNK, _NCHUNK], f32, tag="xsq")
            nc.scalar.dma_start(xsq0[:], xsq_p[0])
            st[0]["xsq"] = xsq0
            cshift = cpool.tile([_CHUNK, 1], f32)
            nc.vector.memset(cshift[:], -_SHIFT)

            # separate psum tiles per image so image 0's counts can drain
            # to HBM while image 1's count mms are still accumulating
            cnt = [cpsum.tile([_CHUNK, 256], f32, tag="cnt", name=f"cnt_{b}")
                   for b in range(_BPC)]

            # HAM warm-up: ~3.4us of zero matmuls ride the boot-DMA wait so
            # the real stream starts at 2.4GHz instead of 1.2GHz; they borrow
            # cnt[0]'s psum region (the real chain restarts it with start=True)
            cwarm = cpool.tile([_D, 256], f16)
            nc.vector.memset(cwarm[:], 0.0)
            for w in range(16):
                nc.tensor.matmul(cnt[0][0:1, 0:256], cwarm[:, w:w + 1],
                                 cwarm[:, 0:256], start=(w == 0), stop=(w == 15))

            def emit_loads(b):
                S = st[b]
                if b != 0:
                    S["xc"] = []
                for g in range(len(_GRPS[b])):
                    emit_load_group(b, g, nc.gpsimd)
                if b != 0:
                    xsq = mpool.tile([_CHUNK, _NCHUNK], f32, tag="xsq")
                    nc.gpsimd.dma_start(xsq[:], xsq_p[b])
                    S["xsq"] = xsq

            def emit_phase1(b, hooks=None, last_pre=None):
                S = st[b]
                m_mat = mpool.tile([_CHUNK, _NCHUNK], f32, tag="m_mat")
                S["m_mat"] = m_mat
                S["sgn"] = []
                for c in range(_NCHUNK):
                    if hooks is not None and c in hooks:
                        hooks[c]()
                    if b == 0 and c < _NBOOT:
                        lo0 = _NBOOT * _CHUNK
                        xh = boot[:, c * _CHUNK:(c + 1) * _CHUNK]
                        xl = boot[:, lo0 + c * _CHUNK:lo0 + (c + 1) * _CHUNK]
                    else:
                        goff, gn, tcb = next(
                            (o, n, t) for o, n, t in S["xc"] if o <= c < o + n
                        )
                        ci = c - goff
                        xh = tcb[:, 0, ci * _CHUNK:(ci + 1) * _CHUNK]
                        xl = tcb[:, 1, ci * _CHUNK:(ci + 1) * _CHUNK]
                    gp = gpsum.tile([_CHUNK, _K], f32, tag="gp")
                    nc.tensor.matmul(gp[:, h0], xh, ch[:, h0], start=True, stop=False)
                    nc.tensor.matmul(gp[:, h1], xh, ch[:, h1], start=True, stop=False)
                    nc.tensor.matmul(gp[:, h0], xh, cl[:, h0], start=False, stop=False)
                    nc.tensor.matmul(gp[:, h1], xh, cl[:, h1], start=False, stop=False)
                    nc.tensor.matmul(gp[:, h0], xl, chpp[:, h0], start=False, stop=True)
                    nc.tensor.matmul(gp[:, h1], xl, chpp[:, h1], start=False, stop=True)
                    # row min (exact fp32)
                    nc.vector.tensor_reduce(
                        m_mat[:, c:c + 1], gp[:], axis=mybir.AxisListType.X, op=Alu.min
                    )
                    if last_pre is not None and c == _NCHUNK - 1:
                        # threshold pre-chain ahead of the last indicator;
                        # that indicator moves to DVE so the chain's sqrt
                        # owns the ACT queue at the tail
                        last_pre()
                    # indicator: 0 at argmin column, -1 elsewhere
                    sp = spool.tile([_CHUNK, _K], f16, tag="sgn")
                    if last_pre is not None and c == _NCHUNK - 1:
                        nc.vector.tensor_scalar(
                            out=sp[:], in0=gp[:], scalar1=m_mat[:, c:c + 1],
                            scalar2=1.0, op0=Alu.is_equal, op1=Alu.subtract,
                        )
                    else:
                        nc.scalar.activation(
                            sp[:], gp[:], Act.Sign,
                            bias=m_mat[:, c:c + 1], scale=-1.0,
                        )
                    S["sgn"].append(sp)

            def emit_phase15_pre(b):
                """Moments without a DVE->ACT->DVE chain: S=sum(min_d) rides
                the ACT sqrt's accumulator, Q=sum(minsq) is pure DVE, and the
                shifted SSQ' = Q - 2c*S + c^2*n is formed per-partition on
                ACT (so no full-magnitude fp32 cancellation)."""
                S = st[b]
                minsq = mpool.tile([_CHUNK, _NCHUNK], f32, tag="minsq")
                nc.vector.tensor_tensor(minsq[:], S["m_mat"][:], S["xsq"][:], op=Alu.add)
                rs2 = mpool.tile([_CHUNK, 2], f32, tag="rs2")
                min_d = mpool.tile([_CHUNK, _NCHUNK], f32, tag="min_d")
                nc.scalar.activation(min_d[:], minsq[:], Act.Sqrt,
                                     accum_out=rs2[:, 0:1])
                # sum of (min_d - c)^2 accumulated directly on ACT: small
                # magnitudes, no cancellation, and no DVE dependency
                sqs = mpool.tile([_CHUNK, _NCHUNK], f32, tag="sqs")
                nc.scalar.activation(sqs[:], min_d[:], Act.Square,
                                     bias=cshift[:], accum_out=rs2[:, 1:2])
                S["min_d"] = min_d
                S["rs2"] = rs2

            def emit_reduce(b):
                S = st[b]
                ar = mpool.tile([_CHUNK, 2], f32, tag="ar")
                nc.gpsimd.partition_all_reduce(ar[:], S["rs2"][:], _CHUNK, bass_isa.ReduceOp.add)
                S["ar"] = ar

            def emit_phase15_post(b):
                """thr computed redundantly on all 128 partitions; mask16.
                sd = sqrt(SSQ'/(N-1) - S'^2/(N(N-1))) with S' = S - c*N;
                mask compare folds the mean: (min_d - sd) < S/N."""
                S = st[b]
                ar = S["ar"]
                spr = mpool.tile([_CHUNK, 1], f32, tag="spr")
                nc.vector.tensor_scalar(
                    out=spr[:], in0=ar[:, 0:1], scalar1=float(_SHIFT * _N),
                    scalar2=None, op0=Alu.subtract,
                )
                mean = mpool.tile([_CHUNK, 1], f32, tag="mean")
                nc.vector.tensor_scalar_mul(mean[:], ar[:, 0:1], 1.0 / _N)
                v1 = mpool.tile([_CHUNK, 1], f32, tag="v1")
                nc.vector.scalar_tensor_tensor(
                    out=v1[:], in0=spr[:], scalar=-1.0 / (_N * (_N - 1.0)),
                    in1=spr[:], op0=Alu.mult, op1=Alu.mult,
                )
                sd = mpool.tile([_CHUNK, 1], f32, tag="sd")
                nc.scalar.activation(sd[:], ar[:, 1:2], Act.Sqrt,
                                     bias=v1[:], scale=1.0 / (_N - 1.0))
                mask16 = mpool.tile([_CHUNK, _NCHUNK], f16, tag="mask16")
                # first columns split out so the count mms' weight loads can
                # start before the full mask finishes
                nc.vector.tensor_scalar(
                    out=mask16[:, 0:4], in0=S["min_d"][:, 0:4], scalar1=sd[:],
                    scalar2=mean[:], op0=Alu.subtract, op1=Alu.is_lt,
                )
                nc.vector.tensor_scalar(
                    out=mask16[:, 4:], in0=S["min_d"][:, 4:], scalar1=sd[:],
                    scalar2=mean[:], op0=Alu.subtract, op1=Alu.is_lt,
                )
                S["mask16"] = mask16

            def emit_counts(b):
                """4-way column-tiled fp16 count matmuls: K-slice j at psum
                partition 32j; image b in cnt cols [256b, 256b+256)."""
                S = st[b]
                for c in range(_NCHUNK):
                    for j in range(4):
                        nc.tensor.matmul(
                            cnt[b][32 * j:32 * j + 1, 0:256],
                            S["mask16"][:, c:c + 1],
                            S["sgn"][c][:, 256 * j:256 * (j + 1)],
                            start=(c == 0), stop=(c == _NCHUNK - 1),
                            tile_position=(0, 32 * j),
                        )

            def emit_out(b):
                cnt_sb = fpool.tile([_CHUNK, 256], f32, tag="cnt_sb")
                nc.vector.tensor_scalar_mul(cnt_sb[:], cnt[b][:], 1.0)
                nc.sync.dma_start(out_p[b], cnt_sb[0:128:32, :])

            # ---- staged emission ----
            emit_loads(0)
            emit_phase1(0)
            emit_phase15_pre(0)
            emit_loads(1)
            emit_reduce(0)      # gpsimd: after img1 dma issues
            emit_phase1(1, hooks={8: lambda: emit_phase15_post(0)},
                        last_pre=lambda: emit_phase15_pre(1))
            emit_reduce(1)
            emit_counts(0)      # PE: covers img1 threshold-chain latency
            emit_phase15_post(1)
            emit_out(0)         # DVE copy + DMA under img1's count mms
            emit_counts(1)
            emit_out(1)

    nc.compile()
    return nc


def _prep_inputs(x, centroids, weights):
    """Host-side layout prep: per-core shards, fp16 hi/lo splits, ||x||^2,
    and the chpp tensor carrying the ||c||^2 fp16 hi/mid rows."""
    x = np.ascontiguousarray(np.asarray(x, dtype=np.float32))
    c = np.asarray(centroids, dtype=np.float32)

    c2 = (-2.0 * c.astype(np.float64).T)              # [D, K]
    ch = c2.astype(np.float16)
    cl = (c2 - ch.astype(np.float64)).astype(np.float16)
    a = (c.astype(np.float64) ** 2).sum(1)            # [K]
    a_hi = a.astype(np.float16)
    a_mid = (a - a_hi.astype(np.float64)).astype(np.float16)
    chpp = ch.copy()
    chpp[_D - 2, :] = a_hi
    chpp[_D - 1, :] = a_mid

    xsq = (x.astype(np.float64) ** 2).sum(-1).astype(np.float32)   # [B, N]

    in_maps = []
    for core in range(_NCORES):
        xs = x[core * _BPC:(core + 1) * _BPC]             # [BPC, N, D]
        xt = np.ascontiguousarray(xs.transpose(0, 2, 1))  # [BPC, D, N]
        xt_hi = xt.astype(np.float16)
        xt_lo = (xt - xt_hi.astype(np.float32)).astype(np.float16)
        xt_lo[:, _D - 2:_D, :] = 1.0                      # a-fold rows
        xt_c = np.ascontiguousarray(np.stack([xt_hi, xt_lo], axis=2))
        nb = _NBOOT * _CHUNK
        boot = np.ascontiguousarray(np.concatenate(
            [xt_hi[0][:, 0:nb], xt_lo[0][:, 0:nb], ch], axis=1
        ))                                                 # [D, 2*nb+K]
        xsq_c = np.ascontiguousarray(
            xsq[core * _BPC:(core + 1) * _BPC]
            .reshape(_BPC, _NCHUNK, _CHUNK).transpose(0, 2, 1)
        )                                                  # [BPC, 128, 32]
        in_maps.append({
            "xt_c": xt_c, "boot": boot, "cl": cl, "cp": chpp, "xsq": xsq_c,
        })
    return in_maps


def kernel(x, centroids, weights, _trace=False, _tmpdir=None):
    from concourse.bass_utils import run_bass_kernel_spmd

    if "nc" not in _compiled:
        _compiled["nc"] = _build()
    nc = _compiled["nc"]

    in_maps = _prep_inputs(x, centroids, weights)
    kw = {}
    if _trace:
        kw = {"trace": True, "tmpdir": _tmpdir}
    res = run_bass_kernel_spmd(nc, in_maps, core_ids=list(range(_NCORES)), **kw)
    cnt = np.concatenate(
        [r["out"].reshape(_BPC, _K) for r in res.results], axis=0
    ).astype(np.float64)                                  # [B, K] = counts - mtot
    # sign matrix sums to (counts_k - mtot) per bin; sum_k cnt = (1-K)*mtot
    mtot = np.rint(-cnt.sum(axis=1, keepdims=True) / (_K - 1))
    counts = np.rint(cnt + mtot)
    asmk = counts * np.asarray(weights, dtype=np.float64)[None, :]
    norm = np.linalg.norm(asmk, axis=1, keepdims=True)
    out = asmk / np.maximum(norm, _EPS)
    if _trace:
        kernel.last_results = res
    return out.astype(np.float32)
